# revision 18
# baseline (speedup 1.0000x reference)
"""EnhancedGAT Trainium2 kernel (8 NeuronCores, Bass/Tile).

Strategy:
  - Host: add self-loops (fill='mean'), sort edges by dst, shard dst-contiguous
    node ranges across 8 cores, build padded per-(node-tile, src-half) edge
    runs + int16 gather indices (dma_gather), upload preformatted constants.
  - Device (per core, SPMD):
      Phase A: h1ext = x @ [W1|v_s1|v_d1]^T for ALL nodes (replicated) ->
               fp16 gather tables T_h1_lo/hi ([h1(256)|a_src1(4)] rows) and a
               local a_dst1 table for this core's nodes.
      L1 edge phase: per node-tile (128 dst nodes): dma_gather h1ext[src] and
               a_dst1[dst]; z = a_src+a_dst+ea*c1; u = exp(leaky(z));
               Msg = u*h; aggregate via selection-matrix matmuls into
               channel-major PSUM [128c x 128n] (+ per-head softmax denom);
               divide.
      BN1 (AllReduce stats) + ReLU + h2ext^T = W2ext^T @ g^T; transpose to
               node-major rows; AllGather the full T_h2 table.
      L2 edge phase: same gather/softmax/aggregate with 1 head, 64 ch.
      BN2 + ReLU; graph pooling via selection matmul over the sorted batch
               vector; AllReduce pooled sums; fc for this core's graph shard.
  - Host: concat the 8 graph-shard outputs.
"""
import sys

sys.path.insert(0, "/opt/trn_rl_repo")

import math
from dataclasses import dataclass, field

import numpy as np

import concourse.bass as bass
import concourse.bacc as bacc
import concourse.mybir as mybir
import concourse.tile as tile
from concourse import bass_utils

F16 = mybir.dt.float16
F32 = mybir.dt.float32
I16 = mybir.dt.int16
I32 = mybir.dt.int32

NEG_SLOPE = 0.2
BN_EPS = 1e-5


@dataclass
class Cfg:
    N: int = 50000
    E: int = 800000
    G: int = 256
    D: int = 8           # cores
    F: int = 128         # input features
    HID: int = 64
    HEADS: int = 4
    LO: int = 32768      # int16 gather table split
    GW: int = 64         # pooling graph window per core
    n_cores: int = 8

    HC: int = field(init=False)
    ND: int = field(init=False)
    NTILES: int = field(init=False)
    NPAD: int = field(init=False)
    NT_ALL: int = field(init=False)
    N_ALL_PAD: int = field(init=False)
    HI_ROWS: int = field(init=False)
    GD: int = field(init=False)

    def __post_init__(self):
        self.HC = self.HEADS * self.HID
        assert self.N % self.D == 0
        self.ND = self.N // self.D
        self.NTILES = math.ceil(self.ND / 128)
        self.NPAD = self.NTILES * 128
        self.NT_ALL = math.ceil(self.N / 128)
        self.N_ALL_PAD = self.NT_ALL * 128
        assert self.LO % 128 == 0 and self.LO < self.N
        self.HI_ROWS = self.N_ALL_PAD - self.LO
        assert self.G % self.D == 0
        self.GD = self.G // self.D


def _pack16(vals, epad):
    """dma_gather idx layout: i -> [i%16, i//16], replicated to 128 rows."""
    arr = np.zeros((16, epad // 16), dtype=np.int16)
    arr[:, : len(vals) // 16 + (1 if len(vals) % 16 else 0)]
    a = np.zeros(epad, dtype=np.int16)
    a[: len(vals)] = vals
    arr = a.reshape(epad // 16, 16).T.copy()
    return np.tile(arr, (8, 1))


def _pack128(vals, epad, dtype):
    a = np.zeros(epad, dtype=dtype)
    a[: len(vals)] = vals
    return a.reshape(epad // 128, 128).T.copy()


def preprocess(cfg: Cfg, inputs: dict):
    N, E, G, D = cfg.N, cfg.E, cfg.G, cfg.D
    ND, NTILES = cfg.ND, cfg.NTILES
    x = np.asarray(inputs["x"], np.float32)
    ei = np.asarray(inputs["edge_index"], np.int64)
    ea = np.asarray(inputs["edge_attr"], np.float32)[:, 0]
    batch = np.asarray(inputs["batch"], np.int64)
    src0, dst0 = ei[0].astype(np.int64), ei[1].astype(np.int64)

    # self loops, fill='mean'
    cnt = np.bincount(dst0, minlength=N).astype(np.float32)
    asum = np.bincount(dst0, weights=ea, minlength=N).astype(np.float32)
    loop_attr = asum / np.maximum(cnt, 1.0)
    src = np.concatenate([src0, np.arange(N, dtype=np.int64)])
    dst = np.concatenate([dst0, np.arange(N, dtype=np.int64)])
    eav = np.concatenate([ea, loop_attr]).astype(np.float32)

    dev = dst // ND
    loc = dst - dev * ND
    tl = loc >> 7
    hi = (src >= cfg.LO).astype(np.int64)

    # group = ((dev*NTILES + tile)*2 + hi)
    grp = (dev * NTILES + tl) * 2 + hi
    order = np.argsort(grp, kind="stable")
    grp_s = grp[order]
    ngroups = D * NTILES * 2
    gcnts = np.bincount(grp_s, minlength=ngroups).reshape(D, NTILES, 2)

    # dummy edges for pad node columns (go to last tile's hi group)
    n_dummy = cfg.NPAD - ND
    gcnts_d = gcnts.copy()
    gcnts_d[:, NTILES - 1, 1] += n_dummy

    # uniform subtile counts across devices
    L = np.maximum(1, np.ceil(gcnts_d.max(axis=0) / 128.0).astype(np.int64))  # [NTILES,2]
    epad = int(L.sum() * 128)
    # base offset of each (tile, half) group (uniform across devices)
    base = np.zeros((NTILES, 2), dtype=np.int64)
    run = np.concatenate([[0], (L * 128).reshape(-1).cumsum()[:-1]])
    base[:, 0] = run[0::2]
    base[:, 1] = run[1::2]

    # position of each (sorted) real edge
    gstart = np.concatenate([[0], np.cumsum(gcnts.reshape(-1))[:-1]])
    rank = np.arange(len(grp_s)) - gstart[grp_s]
    pos = base[tl[order], hi[order]] + rank

    zero_row_hi = N - cfg.LO  # global row N is an x-pad row => h == 0

    per_core = []
    for d in range(D):
        m = dev[order] == d
        o_d = order[m]
        pos_d = pos[m]
        h_idx = np.zeros(epad, dtype=np.int64)
        sd_idx = np.zeros(epad, dtype=np.int64)
        dl = -np.ones(epad, dtype=np.float32)
        eav_d = np.zeros(epad, dtype=np.float32)
        h_val = np.where(src[o_d] >= cfg.LO, src[o_d] - cfg.LO, src[o_d])
        h_idx[pos_d] = h_val
        sd_idx[pos_d] = loc[o_d]
        dl[pos_d] = (loc[o_d] - (tl[o_d] << 7)).astype(np.float32)
        eav_d[pos_d] = eav[o_d]
        # pads: h_idx 0 (valid row), sd_idx 0, dl -1, ea 0 -- already by init,
        # except dl: only positions NOT assigned stay -1. Good.
        # dummy edges for pad node columns: place at end of last tile hi run
        dcnt = int(gcnts[d, NTILES - 1, 1])
        dpos = base[NTILES - 1, 1] + dcnt + np.arange(n_dummy)
        assert dcnt + n_dummy <= L[NTILES - 1, 1] * 128
        h_idx[dpos] = zero_row_hi  # h row == 0
        sd_idx[dpos] = 0
        dl[dpos] = (ND - (NTILES - 1) * 128) + np.arange(n_dummy)
        eav_d[dpos] = 0.0

        assert h_idx.max() < max(cfg.LO, cfg.HI_ROWS) and h_idx.min() >= 0
        assert sd_idx.max() < cfg.NPAD

        # graph window
        g0 = int(min(batch[d * ND], G - cfg.GW, (cfg.GD) * d))
        g0 = max(g0, 0)
        assert int(batch[(d + 1) * ND - 1]) < g0 + cfg.GW
        assert cfg.GD * d >= g0 and cfg.GD * (d + 1) <= g0 + cfg.GW
        bl = batch[d * ND : (d + 1) * ND].astype(np.float32) - g0
        bl = np.concatenate([bl, -np.ones(cfg.NPAD - ND, np.float32)])
        bloc = bl.reshape(NTILES, 128).T.copy()  # [128, NTILES]

        per_core.append(
            dict(
                h_idx=_pack16(h_idx, epad),
                sd_idx=_pack16(sd_idx, epad),
                dstloc=_pack128(dl, epad, np.float16),
                eav=_pack128(eav_d, epad, np.float32),
                bloc=bloc,
                goff=(g0 + np.arange(cfg.GW, dtype=np.int32)).reshape(cfg.GW, 1),
                goff2=(cfg.GD * d + np.arange(cfg.GD, dtype=np.int32)).reshape(
                    cfg.GD, 1
                ),
            )
        )

    gcnt_graph = np.bincount(batch, minlength=G).astype(np.float32)
    recip_gcnt = (1.0 / np.maximum(gcnt_graph, 1.0)).astype(np.float32)
    for d in range(D):
        per_core[d]["recip_gcnt"] = recip_gcnt[cfg.GD * d : cfg.GD * (d + 1)].reshape(
            cfg.GD, 1
        )

    # ---- weights / consts ----
    W1 = np.asarray(inputs["W1"], np.float32)      # [HC, F]
    att_s1 = np.asarray(inputs["att_s1"], np.float32)  # [H, HID]
    att_d1 = np.asarray(inputs["att_d1"], np.float32)
    att_e1 = np.asarray(inputs["att_e1"], np.float32)
    We1 = np.asarray(inputs["We1"], np.float32)    # [HC, 1]
    W2 = np.asarray(inputs["W2"], np.float32)      # [HID, HC]
    att_s2 = np.asarray(inputs["att_s2"], np.float32)  # [1, HID]
    att_d2 = np.asarray(inputs["att_d2"], np.float32)
    att_e2 = np.asarray(inputs["att_e2"], np.float32)
    We2 = np.asarray(inputs["We2"], np.float32)    # [HID, 1]
    H, HID, HC, F = cfg.HEADS, cfg.HID, cfg.HC, cfg.F

    v_s1 = np.einsum("hc,hcf->hf", att_s1, W1.reshape(H, HID, F))  # [H, F]
    v_d1 = np.einsum("hc,hcf->hf", att_d1, W1.reshape(H, HID, F))
    c1 = np.einsum("hc,hc->h", att_e1, We1.reshape(H, HID))        # [H]
    W1extT = np.concatenate([W1.T, v_s1.T, v_d1.T], axis=1)        # [F, HC+2H]
    v_s2 = att_s2 @ W2                                             # [1, HC]
    v_d2 = att_d2 @ W2
    c2 = float(np.einsum("hc,ch->", att_e2, We2))
    W2extT = np.concatenate([W2.T, v_s2.T, v_d2.T], axis=1)        # [HC, HID+2]
    nchh = HC // 128
    W2extT3 = (
        W2extT.reshape(nchh, 128, HID + 2).transpose(1, 0, 2).astype(np.float16)
    )

    xT = np.zeros((F, cfg.N_ALL_PAD), dtype=np.float16)
    xT[:, :N] = x.T.astype(np.float16)

    iota = np.tile(np.arange(128, dtype=np.float16)[None, :], (128, 1))
    E4 = np.zeros((H, HC), dtype=np.float16)
    for k in range(H):
        E4[k, k * HID : (k + 1) * HID] = 1.0
    ones1 = np.ones((1, HID), dtype=np.float16)
    ident = np.eye(128, dtype=np.float16)
    c1rep = np.tile(c1.astype(np.float32)[None, :], (128, 1))

    nheadcols = HC // 128  # channel-major halves (2 for HC=256)
    g1 = np.asarray(inputs["g1"], np.float32).reshape(nheadcols, 128).T.copy()
    be1 = np.asarray(inputs["be1"], np.float32).reshape(nheadcols, 128).T.copy()
    g2 = np.asarray(inputs["g2"], np.float32).reshape(HID, 1)
    be2 = np.asarray(inputs["be2"], np.float32).reshape(HID, 1)
    wfc_rep = np.tile(
        np.asarray(inputs["Wfc"], np.float32).reshape(1, HID), (128, 1)
    )
    bfc = float(np.asarray(inputs["bfc"], np.float32).reshape(-1)[0])

    shared = dict(
        xT=xT,
        W1extT=W1extT.astype(np.float16),
        W2extT=W2extT3,
        c1rep=c1rep,
        iota=iota,
        E4=E4,
        ones1=ones1,
        ident=ident,
        g1=g1,
        be1=be1,
        g2=g2,
        be2=be2,
        wfc_rep=wfc_rep,
    )
    for d in range(D):
        xTl = xT[:, d * ND : (d + 1) * ND]
        xTl = np.concatenate(
            [xTl, np.zeros((F, cfg.NPAD - ND), np.float16)], axis=1
        )
        per_core[d]["xTl"] = xTl
        per_core[d].update(shared)

    sched = dict(L=L, epad=epad, c2=c2, bfc=bfc)
    return per_core, sched


def build(cfg: Cfg, sched) -> tuple:
    """Build the SPMD Bass program. Returns (nc, out_name)."""
    L = sched["L"]
    epad = int(sched["epad"])
    c2 = float(sched["c2"])
    bfc = float(sched["bfc"])
    H, HID, HC, F = cfg.HEADS, cfg.HID, cfg.HC, cfg.F
    NT, NPAD = cfg.NTILES, cfg.NPAD
    EXT1 = HC + 2 * H      # 264
    ROW1 = 384 if HC == 256 else ((HC + 2 * H + 127) // 128) * 128
    ROW2 = 128
    NCH = HC // 128        # chan-major halves in layer-1 (2)
    Lmax = int(L.max())

    nc = bacc.Bacc(
        "TRN2", target_bir_lowering=False, debug=False, num_devices=cfg.n_cores
    )

    def inp(name, shape, dt):
        return nc.dram_tensor(name, list(shape), dt, kind="ExternalInput")

    t_xT = inp("xT", [F, cfg.N_ALL_PAD], F16)
    t_xTl = inp("xTl", [F, NPAD], F16)
    t_hidx = inp("h_idx", [128, epad // 16], I16)
    t_sdidx = inp("sd_idx", [128, epad // 16], I16)
    t_dstloc = inp("dstloc", [128, epad // 128], F16)
    t_eav = inp("eav", [128, epad // 128], F32)
    t_bloc = inp("bloc", [128, NT], F32)
    t_goff = inp("goff", [cfg.GW, 1], I32)
    t_goff2 = inp("goff2", [cfg.GD, 1], I32)
    t_rgc = inp("recip_gcnt", [cfg.GD, 1], F32)
    t_W1extT = inp("W1extT", [F, EXT1], F16)
    t_W2extT = inp("W2extT", [128, NCH, HID + 2], F16)
    t_c1rep = inp("c1rep", [128, H], F32)
    t_iota = inp("iota", [128, 128], F16)
    t_E4 = inp("E4", [H, HC], F16)
    t_ones1 = inp("ones1", [1, HID], F16)
    t_ident = inp("ident", [128, 128], F16)
    t_g1 = inp("g1", [128, NCH], F32)
    t_be1 = inp("be1", [128, NCH], F32)
    t_g2 = inp("g2", [HID, 1], F32)
    t_be2 = inp("be2", [HID, 1], F32)
    t_wfc = inp("wfc_rep", [128, HID], F32)

    t_out = nc.dram_tensor("out_shard", [cfg.GD, 1], F32, kind="ExternalOutput")

    # internal DRAM
    t_h1lo = nc.dram_tensor("T_h1_lo", [cfg.LO, ROW1], F16, kind="Internal")
    t_h1hi = nc.dram_tensor("T_h1_hi", [cfg.HI_ROWS, ROW1], F16, kind="Internal")
    t_ad1 = nc.dram_tensor("T_ad1", [NPAD, ROW2], F16, kind="Internal")
    t_ad2 = nc.dram_tensor("T_ad2", [NPAD, ROW2], F16, kind="Internal")
    t_h2sh = nc.dram_tensor("T_h2_shard", [cfg.ND, ROW2], F16, kind="Internal")
    t_h2f = nc.dram_tensor(
        "T_h2_full", [cfg.N_ALL_PAD, ROW2], F16, kind="Internal", addr_space="Shared"
    )
    t_bn1i = nc.dram_tensor("bn1_in", [128, 2 * NCH], F32, kind="Internal")
    t_bn1o = nc.dram_tensor(
        "bn1_out", [128, 2 * NCH], F32, kind="Internal", addr_space="Shared"
    )
    t_bn2i = nc.dram_tensor("bn2_in", [HID, 2], F32, kind="Internal")
    t_bn2o = nc.dram_tensor(
        "bn2_out", [HID, 2], F32, kind="Internal", addr_space="Shared"
    )
    t_pooli = nc.dram_tensor("pool_in", [cfg.G, HID], F32, kind="Internal")
    t_poolo = nc.dram_tensor(
        "pool_out", [cfg.G, HID], F32, kind="Internal", addr_space="Shared"
    )

    RG = [list(range(cfg.n_cores))]
    AX = mybir.AxisListType
    AF = mybir.ActivationFunctionType
    OP = mybir.AluOpType

    with tile.TileContext(nc) as tc:
        with (
            tc.tile_pool(name="res", bufs=1) as res,
            tc.tile_pool(name="cst", bufs=1) as cst,
        ):
            # resident edge metadata
            hidx_sb = res.tile([128, epad // 16], I16)
            nc.sync.dma_start(hidx_sb[:], t_hidx[:])
            sdidx_sb = res.tile([128, epad // 16], I16)
            nc.sync.dma_start(sdidx_sb[:], t_sdidx[:])
            dstloc_sb = res.tile([128, epad // 128], F16)
            nc.sync.dma_start(dstloc_sb[:], t_dstloc[:])
            eav_sb = res.tile([128, epad // 128], F32)
            nc.sync.dma_start(eav_sb[:], t_eav[:])
            bloc_sb = res.tile([128, NT], F32)
            nc.sync.dma_start(bloc_sb[:], t_bloc[:])

            # consts
            W1extT_sb = cst.tile([F, EXT1], F16)
            nc.sync.dma_start(W1extT_sb[:], t_W1extT[:])
            W2extT_sb = cst.tile([128, NCH, HID + 2], F16)
            nc.sync.dma_start(W2extT_sb[:], t_W2extT[:])
            c1rep_sb = cst.tile([128, H], F32)
            nc.sync.dma_start(c1rep_sb[:], t_c1rep[:])
            iota_sb = cst.tile([128, 128], F16)
            nc.sync.dma_start(iota_sb[:], t_iota[:])
            E4_sb = cst.tile([H, HC], F16)
            nc.sync.dma_start(E4_sb[:], t_E4[:])
            ones1_sb = cst.tile([1, HID], F16)
            nc.sync.dma_start(ones1_sb[:], t_ones1[:])
            ident_sb = cst.tile([128, 128], F16)
            nc.sync.dma_start(ident_sb[:], t_ident[:])
            g1_sb = cst.tile([128, NCH], F32)
            nc.sync.dma_start(g1_sb[:], t_g1[:])
            be1_sb = cst.tile([128, NCH], F32)
            nc.sync.dma_start(be1_sb[:], t_be1[:])
            g2_sb = cst.tile([HID, 1], F32)
            nc.sync.dma_start(g2_sb[:], t_g2[:])
            be2_sb = cst.tile([HID, 1], F32)
            nc.sync.dma_start(be2_sb[:], t_be2[:])
            wfc_sb = cst.tile([128, HID], F32)
            nc.sync.dma_start(wfc_sb[:], t_wfc[:])
            goff_sb = cst.tile([cfg.GW, 1], I32)
            nc.sync.dma_start(goff_sb[:], t_goff[:])
            goff2_sb = cst.tile([cfg.GD, 1], I32)
            nc.sync.dma_start(goff2_sb[:], t_goff2[:])
            rgc_sb = cst.tile([cfg.GD, 1], F32)
            nc.sync.dma_start(rgc_sb[:], t_rgc[:])
            zconst_sb = cst.tile([128, 1], F32)
            nc.vector.memset(zconst_sb[:], 0.0)
            econst_sb = cst.tile([128, 1], F32)
            nc.vector.memset(econst_sb[:], BN_EPS)
            nc.const_aps.aps[(F32, 0.0)] = zconst_sb[:]
            nc.const_aps.aps[(F32, BN_EPS)] = econst_sb[:]

            # out1T / later tensors, resident
            out1T = [res.tile([128, NPAD], F16, name=f"out1T{i}") for i in range(NCH)]
            gT = [res.tile([128, NPAD], F16, name=f"gT{i}") for i in range(NCH)]
            out2T = res.tile([HID, NPAD], F16)
            g2T = res.tile([HID, NPAD], F16)

            # ---------------- Phase A ----------------
            with (
                tc.tile_pool(name="pa_sb", bufs=3) as pa,
                tc.tile_pool(name="pa_ps", bufs=4, space="PSUM") as paps,
            ):
                for t in range(cfg.NT_ALL):
                    xt = pa.tile([128, 128], F16, tag="xt")
                    nc.sync.dma_start(xt[:], t_xT[:, t * 128 : (t + 1) * 128])
                    ps = paps.tile([128, EXT1], F32)
                    nc.tensor.matmul(
                        ps[:], lhsT=xt[:], rhs=W1extT_sb[:], start=True, stop=True
                    )
                    st = pa.tile([128, HC + H], F16, tag="st")
                    nc.vector.tensor_copy(st[:, : HC // 2], ps[:, : HC // 2])
                    nc.scalar.copy(st[:, HC // 2 :], ps[:, HC // 2 : HC + H])
                    if t * 128 < cfg.LO:
                        dst_ap = t_h1lo[t * 128 : (t + 1) * 128, : HC + H]
                    else:
                        r0 = t * 128 - cfg.LO
                        dst_ap = t_h1hi[r0 : r0 + 128, : HC + H]
                    nc.sync.dma_start(dst_ap, st[:])
                # local a_dst1 pass
                adst_stage = pa.tile([128, NT, H], F16, tag="adst")
                for t in range(NT):
                    xt = pa.tile([128, 128], F16, tag="xt")
                    nc.sync.dma_start(xt[:], t_xTl[:, t * 128 : (t + 1) * 128])
                    ps2 = paps.tile([128, H], F32, tag="ps2")
                    nc.tensor.matmul(
                        ps2[:],
                        lhsT=xt[:],
                        rhs=W1extT_sb[:, HC + H : HC + 2 * H],
                        start=True,
                        stop=True,
                    )
                    nc.scalar.copy(adst_stage[:, t, :], ps2[:])
                # write all rows: row r=128*i+p, cols 0:H
                nc.sync.dma_start(
                    t_ad1[:, :H].rearrange("(i p) h -> p i h", p=128), adst_stage[:]
                )

            # ---------------- L1 edge phase ----------------
            def edge_phase(layer):
                """layer 1: tables (t_h1lo, t_h1hi, t_ad1), ROW1, channels HC,
                H heads -> writes out1T halves.
                layer 2: tables (t_h2f slices, t_ad2), ROW2, HID ch, 1 head
                -> writes out2T."""
                if layer == 1:
                    row, nch_cols = ROW1, HC
                    lo_ap, hi_ap = t_h1lo[:], t_h1hi[:]
                    adt = t_ad1
                else:
                    row, nch_cols = ROW2, HID
                    lo_ap = t_h2f[0 : cfg.LO, :]
                    hi_ap = t_h2f[cfg.LO : cfg.N_ALL_PAD, :]
                    adt = t_ad2
                with (
                    tc.tile_pool(name=f"ge{layer}", bufs=2) as ge,
                    tc.tile_pool(name=f"ms{layer}", bufs=2) as ms,
                    tc.tile_pool(name=f"ps{layer}", bufs=2, space="PSUM") as pps,
                    tc.tile_pool(name=f"po{layer}", bufs=2) as po,
                ):
                    off = 0
                    for t in range(NT):
                        if layer == 1:
                            accs = [
                                pps.tile([128, 128], F32, name=f"acc{i}", tag=f"acc{i}")
                                for i in range(NCH)
                            ]
                            accS = pps.tile([H, 128], F32, tag="accS")
                        else:
                            acc2 = pps.tile([HID + 1, 128], F32, tag="acc2")
                        first = True
                        nsub_t = int(L[t, 0] + L[t, 1])
                        done = 0
                        for half in range(2):
                            Lh = int(L[t, half])
                            tab = lo_ap if half == 0 else hi_ap
                            gh = ge.tile([128, Lmax, row], F16, tag="gh")
                            nc.gpsimd.dma_gather(
                                out_ap=gh[:, :Lh, :],
                                in_ap=tab,
                                idxs_ap=hidx_sb[:, off // 16 : (off + Lh * 128) // 16],
                                num_idxs=Lh * 128,
                                num_idxs_reg=Lh * 128,
                                elem_size=row,
                                single_packet=(Lh * 128 <= 1024),
                            )
                            gd = ge.tile([128, Lmax, ROW2], F16, tag="gd")
                            nc.gpsimd.dma_gather(
                                out_ap=gd[:, :Lh, :],
                                in_ap=adt[:],
                                idxs_ap=sdidx_sb[:, off // 16 : (off + Lh * 128) // 16],
                                num_idxs=Lh * 128,
                                num_idxs_reg=Lh * 128,
                                elem_size=ROW2,
                                single_packet=(Lh * 128 <= 1024),
                            )
                            co = off // 128  # column offset into per-128 arrays
                            # z pipeline (batched over the run)
                            if layer == 1:
                                zz = ms.tile([128, Lmax, H], F32, tag="zz")
                                z = zz[:, :Lh, :]
                                nc.vector.tensor_tensor(
                                    out=z,
                                    in0=gh[:, :Lh, HC : HC + H],
                                    in1=gd[:, :Lh, 0:H],
                                    op=OP.add,
                                )
                                tmp = ms.tile([128, Lmax, H], F32, tag="tmp")
                                nc.vector.tensor_tensor(
                                    out=tmp[:, :Lh, :],
                                    in0=eav_sb[:, co : co + Lh].broadcast_to(
                                        [128, Lh, H]
                                    ),
                                    in1=c1rep_sb[:][:, None, :].broadcast_to(
                                        [128, Lh, H]
                                    ),
                                    op=OP.mult,
                                )
                                nc.vector.tensor_tensor(
                                    out=z, in0=z, in1=tmp[:, :Lh, :], op=OP.add
                                )
                            else:
                                zz = ms.tile([128, Lmax], F32, tag="zz")
                                z = zz[:, :Lh]
                                nc.vector.tensor_tensor(
                                    out=z,
                                    in0=gh[:, :Lh, HID],
                                    in1=gd[:, :Lh, 0],
                                    op=OP.add,
                                )
                                tmp = ms.tile([128, Lmax], F32, tag="tmp")
                                nc.vector.tensor_scalar(
                                    out=tmp[:, :Lh],
                                    in0=eav_sb[:, co : co + Lh],
                                    scalar1=c2,
                                    scalar2=None,
                                    op0=OP.mult,
                                )
                                nc.vector.tensor_tensor(
                                    out=z, in0=z, in1=tmp[:, :Lh], op=OP.add
                                )
                            lk = ms.tile(
                                [128, Lmax, H] if layer == 1 else [128, Lmax],
                                F32,
                                tag="lk",
                            )
                            lkv = lk[:, :Lh, :] if layer == 1 else lk[:, :Lh]
                            nc.vector.tensor_scalar(
                                out=lkv,
                                in0=z,
                                scalar1=NEG_SLOPE,
                                scalar2=None,
                                op0=OP.mult,
                            )
                            nc.vector.tensor_tensor(out=z, in0=z, in1=lkv, op=OP.max)
                            uu = ms.tile(
                                [128, Lmax, H] if layer == 1 else [128, Lmax],
                                F16,
                                tag="uu",
                            )
                            uv = uu[:, :Lh, :] if layer == 1 else uu[:, :Lh]
                            nc.scalar.activation(uv, z, AF.Exp)
                            # premul
                            if layer == 1:
                                msg = ms.tile([128, Lmax, HC], F16, tag="msg")
                                nc.vector.tensor_tensor(
                                    out=msg[:, :Lh, :].rearrange(
                                        "p l (h c) -> p l h c", h=H
                                    ),
                                    in0=gh[:, :Lh, 0:HC].rearrange(
                                        "p l (h c) -> p l h c", h=H
                                    ),
                                    in1=uu[:, :Lh, :].broadcast_to(
                                        [128, Lh, H, HID]
                                    ),
                                    op=OP.mult,
                                )
                            else:
                                msg = ms.tile([128, Lmax, HID + 1], F16, tag="msg")
                                nc.vector.tensor_tensor(
                                    out=msg[:, :Lh, 0:HID],
                                    in0=gh[:, :Lh, 0:HID],
                                    in1=uu[:, :Lh].broadcast_to([128, Lh, HID]),
                                    op=OP.mult,
                                )
                                nc.vector.tensor_copy(
                                    msg[:, :Lh, HID], uu[:, :Lh]
                                )
                            # selection matrices + matmuls per subtile
                            sel = ms.tile([128, Lmax, 128], F16, tag="sel")
                            nc.vector.tensor_tensor(
                                out=sel[:, :Lh, :],
                                in0=iota_sb[:][:, None, :].broadcast_to(
                                    [128, Lh, 128]
                                ),
                                in1=dstloc_sb[:, co : co + Lh].broadcast_to(
                                    [128, Lh, 128]
                                ),
                                op=OP.is_equal,
                            )
                            for s in range(Lh):
                                last = done + s == nsub_t - 1
                                if layer == 1:
                                    for i in range(NCH):
                                        nc.tensor.matmul(
                                            accs[i][:],
                                            lhsT=msg[:, s, i * 128 : (i + 1) * 128],
                                            rhs=sel[:, s, :],
                                            start=first,
                                            stop=last,
                                        )
                                    nc.tensor.matmul(
                                        accS[:],
                                        lhsT=uu[:, s, :],
                                        rhs=sel[:, s, :],
                                        start=first,
                                        stop=last,
                                    )
                                else:
                                    nc.tensor.matmul(
                                        acc2[:],
                                        lhsT=msg[:, s, :],
                                        rhs=sel[:, s, :],
                                        start=first,
                                        stop=last,
                                    )
                                first = False
                            done += Lh
                            off += Lh * 128
                        # post: divide by softmax denom
                        if layer == 1:
                            rec = po.tile([H, 128], F16, tag="rec")
                            with nc.allow_low_precision(reason="softmax denom fp16"):
                                nc.vector.reciprocal(rec[:], accS[:])
                            for i in range(NCH):
                                rexp = pps.tile([128, 128], F32, tag="rexp")
                                nc.tensor.matmul(
                                    rexp[:],
                                    lhsT=E4_sb[:, i * 128 : (i + 1) * 128],
                                    rhs=rec[:], start=True, stop=True,
                                )
                                rexs = po.tile([128, 128], F16, tag="rexs")
                                nc.scalar.copy(rexs[:], rexp[:])
                                nc.vector.tensor_tensor(
                                    out=out1T[i][:, t * 128 : (t + 1) * 128],
                                    in0=accs[i][:],
                                    in1=rexs[:],
                                    op=OP.mult,
                                )
                        else:
                            rec = po.tile([1, 128], F16, tag="rec")
                            with nc.allow_low_precision(reason="softmax denom fp16"):
                                nc.vector.reciprocal(rec[:], acc2[HID : HID + 1, :])
                            rexp = pps.tile([HID, 128], F32, tag="rexp")
                            nc.tensor.matmul(
                                rexp[:], lhsT=ones1_sb[:], rhs=rec[:], start=True,
                                stop=True,
                            )
                            rexs = po.tile([HID, 128], F16, tag="rexs")
                            nc.scalar.copy(rexs[:], rexp[:])
                            nc.vector.tensor_tensor(
                                out=out2T[:, t * 128 : (t + 1) * 128],
                                in0=acc2[0:HID, :],
                                in1=rexs[:],
                                op=OP.mult,
                            )

            edge_phase(1)

            # ---------------- BN1 + ReLU + h2ext + AllGather ----------------
            with (
                tc.tile_pool(name="bn1", bufs=1) as bn,
                tc.tile_pool(name="bn1ps", bufs=4, space="PSUM") as bnps,
                tc.tile_pool(name="dram1", bufs=1, space="DRAM") as dr1,
            ):
                stats = bn.tile([128, 2 * NCH], F32)
                scratch = bn.tile([128, NPAD], F16)
                for i in range(NCH):
                    nc.scalar.activation(
                        scratch[:], out1T[i][:], AF.Copy,
                        accum_out=stats[:, 2 * i : 2 * i + 1],
                    )
                    nc.scalar.activation(
                        scratch[:], out1T[i][:], AF.Square,
                        accum_out=stats[:, 2 * i + 1 : 2 * i + 2],
                    )
                nc.sync.dma_start(t_bn1i[:], stats[:])
                nc.gpsimd.collective_compute(
                    "AllReduce", OP.add, replica_groups=RG,
                    ins=[t_bn1i[:]], outs=[t_bn1o[:]],
                )
                rstats = bn.tile([128, 2 * NCH], F32)
                nc.sync.dma_start(rstats[:], t_bn1o[:])
                # mu = s/N ; var = ss/N - mu^2 ; scale = g1*rsqrt(var+eps)
                # beta' = be1 - mu*scale
                mu = bn.tile([128, NCH], F32)
                var = bn.tile([128, NCH], F32)
                scal = bn.tile([128, NCH], F32)
                beta = bn.tile([128, NCH], F32)
                rview = rstats[:].rearrange("p (c two) -> p c two", two=2)
                nc.vector.tensor_scalar(
                    out=mu[:], in0=rview[:, :, 0],
                    scalar1=1.0 / cfg.N, scalar2=None, op0=OP.mult,
                )
                nc.vector.tensor_scalar(
                    out=var[:], in0=rview[:, :, 1],
                    scalar1=1.0 / cfg.N, scalar2=None, op0=OP.mult,
                )
                mu2 = bn.tile([128, NCH], F32)
                nc.vector.tensor_tensor(out=mu2[:], in0=mu[:], in1=mu[:], op=OP.mult)
                nc.vector.tensor_tensor(out=var[:], in0=var[:], in1=mu2[:], op=OP.subtract)
                sd = bn.tile([128, NCH], F32)
                nc.scalar.activation(sd[:], var[:], AF.Sqrt, bias=BN_EPS)
                nc.vector.reciprocal(sd[:], sd[:])
                nc.vector.tensor_tensor(out=scal[:], in0=g1_sb[:], in1=sd[:], op=OP.mult)
                nc.vector.tensor_tensor(out=beta[:], in0=mu[:], in1=scal[:], op=OP.mult)
                nc.vector.tensor_tensor(out=beta[:], in0=be1_sb[:], in1=beta[:], op=OP.subtract)
                for i in range(NCH):
                    nc.scalar.activation(
                        gT[i][:], out1T[i][:], AF.Relu,
                        scale=scal[:, i : i + 1], bias=beta[:, i : i + 1],
                    )
                # h2ext^T per node tile -> transpose -> write shard + T_ad2
                h2stage = bn.tile([128, NT, HID + 2], F16)
                for t in range(NT):
                    psH = bnps.tile([HID + 2, 128], F32, tag="psH")
                    for i in range(NCH):
                        nc.tensor.matmul(
                            psH[:],
                            lhsT=W2extT_sb[:, i, :],
                            rhs=gT[i][:, t * 128 : (t + 1) * 128],
                            start=(i == 0),
                            stop=(i == NCH - 1),
                        )
                    hTs = bn.tile([HID + 2, 128], F16, tag="hTs")
                    nc.scalar.copy(hTs[:], psH[:])
                    psT = bnps.tile([128, HID + 2], F16, tag="psT")
                    nc.tensor.transpose(
                        psT[:], hTs[:], ident_sb[: HID + 2, : HID + 2]
                    )
                    nc.scalar.copy(h2stage[:, t, :], psT[:])
                # write shard rows [ND, 0:HID+1] and T_ad2 rows col HID+1
                nfull = (cfg.ND // 128) * 128
                nc.sync.dma_start(
                    t_h2sh[:nfull, : HID + 1].rearrange("(i p) h -> p i h", p=128),
                    h2stage[:, : cfg.ND // 128, : HID + 1],
                )
                rem = cfg.ND - (cfg.ND // 128) * 128
                if rem:
                    i0 = cfg.ND // 128
                    nc.sync.dma_start(
                        t_h2sh[i0 * 128 :, : HID + 1].rearrange(
                            "(i p) h -> p i h", p=rem
                        ),
                        h2stage[:rem, i0 : i0 + 1, : HID + 1],
                    )
                nc.sync.dma_start(
                    t_ad2[:, 0:1].rearrange("(i p) h -> p i h", p=128),
                    h2stage[:, :, HID + 1 : HID + 2],
                )
                nc.gpsimd.collective_compute(
                    "AllGather", OP.bypass, replica_groups=RG,
                    ins=[t_h2sh[:]], outs=[t_h2f[0 : cfg.N, :]],
                )
                # zero the pad rows of T_h2_full
                zt = bn.tile([cfg.N_ALL_PAD - cfg.N, ROW2], F16)
                nc.vector.memset(zt[:], 0.0)
                nc.sync.dma_start(t_h2f[cfg.N : cfg.N_ALL_PAD, :], zt[:])

            # ---------------- L2 edge phase ----------------
            edge_phase(2)

            # ---------------- BN2 + ReLU + pool + fc ----------------
            with (
                tc.tile_pool(name="bn2", bufs=1) as bn,
                tc.tile_pool(name="bn2ps", bufs=4, space="PSUM") as bnps,
            ):
                stats = bn.tile([HID, 2], F32)
                scratch = bn.tile([HID, NPAD], F16)
                nc.scalar.activation(
                    scratch[:], out2T[:], AF.Copy, accum_out=stats[:, 0:1]
                )
                nc.scalar.activation(
                    scratch[:], out2T[:], AF.Square, accum_out=stats[:, 1:2]
                )
                nc.sync.dma_start(t_bn2i[:], stats[:])
                nc.gpsimd.collective_compute(
                    "AllReduce", OP.add, replica_groups=RG,
                    ins=[t_bn2i[:]], outs=[t_bn2o[:]],
                )
                rstats = bn.tile([HID, 2], F32)
                nc.sync.dma_start(rstats[:], t_bn2o[:])
                mu = bn.tile([HID, 1], F32)
                var = bn.tile([HID, 1], F32)
                nc.vector.tensor_scalar(
                    out=mu[:], in0=rstats[:, 0:1], scalar1=1.0 / cfg.N,
                    scalar2=None, op0=OP.mult,
                )
                nc.vector.tensor_scalar(
                    out=var[:], in0=rstats[:, 1:2], scalar1=1.0 / cfg.N,
                    scalar2=None, op0=OP.mult,
                )
                mu2 = bn.tile([HID, 1], F32)
                nc.vector.tensor_tensor(out=mu2[:], in0=mu[:], in1=mu[:], op=OP.mult)
                nc.vector.tensor_tensor(out=var[:], in0=var[:], in1=mu2[:], op=OP.subtract)
                sd = bn.tile([HID, 1], F32)
                nc.scalar.activation(sd[:], var[:], AF.Sqrt, bias=BN_EPS)
                nc.vector.reciprocal(sd[:], sd[:])
                scal = bn.tile([HID, 1], F32)
                beta = bn.tile([HID, 1], F32)
                nc.vector.tensor_tensor(out=scal[:], in0=g2_sb[:], in1=sd[:], op=OP.mult)
                nc.vector.tensor_tensor(out=beta[:], in0=mu[:], in1=scal[:], op=OP.mult)
                nc.vector.tensor_tensor(out=beta[:], in0=be2_sb[:], in1=beta[:], op=OP.subtract)
                nc.scalar.activation(
                    g2T[:], out2T[:], AF.Relu, scale=scal[:, 0:1], bias=beta[:, 0:1]
                )
                # pooling
                poolAcc = bnps.tile([cfg.GW, HID], F32, tag="poolacc")
                for t in range(NT):
                    psG = bnps.tile([128, HID], F16, tag="psG")
                    nc.tensor.transpose(
                        psG[:], g2T[:, t * 128 : (t + 1) * 128],
                        ident_sb[:HID, :HID],
                    )
                    gnm = bn.tile([128, HID], F16, tag="gnm")
                    nc.scalar.copy(gnm[:], psG[:])
                    selg = bn.tile([128, cfg.GW], F16, tag="selg")
                    nc.vector.tensor_scalar(
                        out=selg[:], in0=iota_sb[:, : cfg.GW],
                        scalar1=bloc_sb[:, t : t + 1], scalar2=None,
                        op0=OP.is_equal,
                    )
                    nc.tensor.matmul(
                        poolAcc[:], lhsT=selg[:], rhs=gnm[:],
                        start=(t == 0), stop=(t == NT - 1),
                    )
                pool_sb = bn.tile([cfg.GW, HID], F32)
                nc.scalar.copy(pool_sb[:], poolAcc[:])
                # zero full pool bounce, scatter window rows
                zp = bn.tile([128, HID], F32)
                nc.vector.memset(zp[:], 0.0)
                for r0 in range(0, cfg.G, 128):
                    rr = min(128, cfg.G - r0)
                    nc.sync.dma_start(t_pooli[r0 : r0 + rr, :], zp[:rr, :])
                nc.gpsimd.indirect_dma_start(
                    out=t_pooli[:],
                    out_offset=bass.IndirectOffsetOnAxis(ap=goff_sb[:, 0:1], axis=0),
                    in_=pool_sb[:],
                    in_offset=None,
                )
                nc.gpsimd.collective_compute(
                    "AllReduce", OP.add, replica_groups=RG,
                    ins=[t_pooli[:]], outs=[t_poolo[:]],
                )
                myp = bn.tile([cfg.GD, HID], F32)
                nc.gpsimd.indirect_dma_start(
                    out=myp[:],
                    out_offset=None,
                    in_=t_poolo[:],
                    in_offset=bass.IndirectOffsetOnAxis(ap=goff2_sb[:, 0:1], axis=0),
                )
                nc.vector.tensor_scalar(
                    out=myp[:], in0=myp[:], scalar1=rgc_sb[:, 0:1], scalar2=None,
                    op0=OP.mult,
                )
                fcm = bn.tile([cfg.GD, HID], F32)
                nc.vector.tensor_tensor(
                    out=fcm[:], in0=myp[:], in1=wfc_sb[: cfg.GD, :], op=OP.mult
                )
                ored = bn.tile([cfg.GD, 1], F32)
                nc.vector.tensor_reduce(ored[:], fcm[:], axis=AX.X, op=OP.add)
                nc.vector.tensor_scalar(
                    out=ored[:], in0=ored[:], scalar1=bfc, scalar2=None, op0=OP.add
                )
                nc.sync.dma_start(t_out[:], ored[:])

    nc.compile()
    return nc, "out_shard"


_CACHE = {}


def _get_program(cfg, sched):
    key = (cfg.N, cfg.E, sched["epad"], tuple(sched["L"].reshape(-1)))
    if key not in _CACHE:
        _CACHE[key] = build(cfg, sched)
    return _CACHE[key]


def kernel(**inputs) -> np.ndarray:
    cfg = Cfg()
    per_core, sched = preprocess(cfg, inputs)
    nc, out_name = _get_program(cfg, sched)
    in_names = {
        "xT", "xTl", "h_idx", "sd_idx", "dstloc", "eav", "bloc", "goff",
        "goff2", "recip_gcnt", "W1extT", "W2extT", "c1rep", "iota", "E4",
        "ones1", "ident", "g1", "be1", "g2", "be2", "wfc_rep",
    }
    name_map = {
        "h_idx": "h_idx", "sd_idx": "sd_idx", "dstloc": "dstloc", "eav": "eav",
        "bloc": "bloc", "goff": "goff", "goff2": "goff2",
        "recip_gcnt": "recip_gcnt", "xT": "xT", "xTl": "xTl",
        "W1extT": "W1extT", "W2extT": "W2extT", "c1rep": "c1rep",
        "iota": "iota", "E4": "E4", "ones1": "ones1", "ident": "ident",
        "g1": "g1", "be1": "be1", "g2": "g2", "be2": "be2",
        "wfc_rep": "wfc_rep",
    }
    in_maps = []
    for d in range(cfg.D):
        m = {}
        for k in in_names:
            m[k] = np.ascontiguousarray(per_core[d][name_map[k]])
        in_maps.append(m)
    res = bass_utils.run_bass_kernel_spmd(
        nc, in_maps, core_ids=list(range(cfg.n_cores))
    )
    out = np.concatenate(
        [np.asarray(res.results[d][out_name]) for d in range(cfg.D)], axis=0
    )
    return out.astype(np.float32)
